# revision 17
# baseline (speedup 1.0000x reference)
"""Trainium2 Bass kernel for the ReLU-RNN problem (nn_RNN).

Math (per core, data-parallel over batch; 8 batch rows per core):
    xp  = x @ W_ih.T + b_ih + b_hh
    h_t = relu(xp_t + h_{t-1} @ W_hh.T)     (S sequential steps)

All recurrent state lives transposed: h^T is [H=512 (4 partition
chunks), B_local=8].  Each step runs 4 chunk-groups: an identity
matmul injects xp_t into a dedicated PSUM bank, 4 bf16 W_hh tile
matmuls accumulate in ROTATED k-order (chunk c starts at k=c, which
makes the relu-latency constraint uniform across chunks), and one DVE
max writes the bf16 h^T staging for the next step.

The input projection (x_proj) is interleaved into the loop: one
(batch-row, 128-timestep) block per 16-step window, emitted after the
window so the Tile scheduler uses it as PE gap-filler while the
recurrence waits on relu semaphores.  x^T tiles come from DMA
transpose, not the PE.  Output windows are finalized inline the same
way (DMA transpose + ScalarE cast + DMA), so nothing interrupts the
steady state.
"""

import numpy as np

import concourse.bass as bass
import concourse.mybir as mybir
from concourse import masks, tile
from concourse.bass_utils import run_bass_kernel_spmd

B, S, I, H = 64, 1024, 512, 512
NCORES = 8
BL = B // NCORES          # batch rows per core
KT = I // 128             # contraction tiles (4)
MT = H // 128             # output-row tiles (4)
WIN = 16                  # recurrence steps per output window
XTC = 128                 # x_proj timesteps per interleaved block
F32 = mybir.dt.float32
BF16 = mybir.dt.bfloat16

# This walrus build rejects instructions carrying more than one sync-wait
# command.  After Tile scheduling, split any excess waits onto same-engine
# NOP instructions inserted immediately before the offending instruction.
_MAX_WAITS = 1


def _split_multi_waits(nc):
    cnt = 0
    for bb in nc.main_func.blocks:
        insts = bb.instructions
        i = 0
        while i < len(insts):
            inst = insts[i]
            si = getattr(inst, "sync_info", None)
            if si is not None and len(si.on_wait) > _MAX_WAITS:
                waits = list(si.on_wait)
                si.on_wait = waits[-_MAX_WAITS:]
                rest = waits[:-_MAX_WAITS]
                for j in range(0, len(rest), _MAX_WAITS):
                    nop = mybir.InstNoOp(
                        name=f"ws_{cnt}",
                        engine=inst.engine,
                        sync_info=mybir.SyncInfo(
                            on_wait=rest[j : j + _MAX_WAITS], on_update=[]
                        ),
                        bass_nofuse=True,
                    )
                    cnt += 1
                    insts.insert(i, nop)
                    i += 1
            i += 1
    return cnt


def build_rnn_kernel(seq_len=S):
    SL = seq_len
    NW = SL // WIN
    NXB = SL // XTC          # x_proj 128-step chunks
    assert SL % WIN == 0 and SL % XTC == 0
    nc = bass.Bass(target_bir_lowering=False, trn_type="TRN2")

    x_d = nc.dram_tensor("inputs", [BL, SL, I], F32, kind="ExternalInput")
    h0_d = nc.dram_tensor("h0", [BL, H], F32, kind="ExternalInput")
    wih_d = nc.dram_tensor("weight_ih", [H, I], F32, kind="ExternalInput")
    whh_d = nc.dram_tensor("weight_hh", [H, H], F32, kind="ExternalInput")
    bih_d = nc.dram_tensor("bias_ih", [H], F32, kind="ExternalInput")
    bhh_d = nc.dram_tensor("bias_hh", [H], F32, kind="ExternalInput")
    out_d = nc.dram_tensor("outputs", [BL, SL, H], F32, kind="ExternalOutput")
    hf_d = nc.dram_tensor("h_final", [BL, H], F32, kind="ExternalOutput")

    with tile.TileContext(nc) as tc:
        with tc.tile_pool(name="const", bufs=1) as constp:
            ident = constp.tile([128, 128], BF16)
            masks.make_identity(nc, ident[:])

            # bias_t[p, hc] = (b_ih + b_hh)[hc*128 + p]
            bias_t = constp.tile([128, MT], F32)
            btmp = constp.tile([128, MT], F32)
            nc.sync.dma_start(bias_t[:], bih_d.rearrange("(hc p) -> p hc", p=128))
            nc.sync.dma_start(btmp[:], bhh_d.rearrange("(hc p) -> p hc", p=128))
            nc.vector.tensor_add(bias_t[:], bias_t[:], btmp[:])

            # lhsT weight tiles: lhsT(kc, mc)[k, m] = W[mc*128+m, kc*128+k]
            wih_T = constp.tile([128, KT * MT * 128], BF16)   # col idx (kc*MT+mc)
            whh_T = constp.tile([128, KT * MT * 128], BF16)
            # xp^T, SBUF-resident bf16, col idx (t, hc, b)
            xp_t = constp.tile([128, SL * MT * BL], BF16)
            # initial h^T staging window (holds h0 in slot WIN-1), idx (hc,t,b)
            stag_init = constp.tile([128, WIN * MT * BL], BF16)

            with (
                tc.tile_pool(name="wld", bufs=2) as wldp,
                tc.tile_pool(name="wtp", bufs=2, space="PSUM") as wtpp,
            ):
                for w_d, w_T in ((wih_d, wih_T), (whh_d, whh_T)):
                    for mc in range(MT):
                        wld = wldp.tile([128, 512], F32, tag="wld")
                        wldb = wldp.tile([128, 512], BF16, tag="wldb")
                        nc.sync.dma_start(wld[:], w_d[mc * 128 : (mc + 1) * 128, :])
                        nc.vector.tensor_copy(wldb[:], wld[:])
                        for kc in range(KT):
                            ps = wtpp.tile([128, 128], BF16, tag="wtp")
                            nc.tensor.transpose(
                                ps[:], wldb[:, kc * 128 : (kc + 1) * 128], ident[:]
                            )
                            j = (kc * MT + mc) * 128
                            nc.vector.tensor_copy(w_T[:, j : j + 128], ps[:])

                # h0 -> bf16, transposed into stag_init slot WIN-1
                h0l = wldp.tile([BL, H], F32, tag="h0l")
                h0b = wldp.tile([BL, H], BF16, tag="h0b")
                nc.sync.dma_start(h0l[:], h0_d[:, :])
                nc.vector.tensor_copy(h0b[:], h0l[:])
                for hc in range(MT):
                    ps = wtpp.tile([128, 128], BF16, tag="wtp")
                    nc.tensor.transpose(
                        ps[:, :BL], h0b[:, hc * 128 : (hc + 1) * 128],
                        ident[:BL, :BL],
                    )
                    off = hc * WIN * BL + (WIN - 1) * BL
                    nc.vector.tensor_copy(
                        stag_init[:, off : off + BL], ps[:, :BL]
                    )

            # view of xp^T as [p, t, hc, b]
            xp_v = xp_t[:].rearrange("p (t hc b) -> p t hc b", hc=MT, b=BL)

            with (
                tc.tile_pool(name="xld", bufs=3) as xldp,
                tc.tile_pool(name="xtr", bufs=3) as xtrp,
            ):
                def xproj_block(b, c, psp, ptag, pbufs):
                    """x_proj for batch row b, timesteps [c*XTC, (c+1)*XTC)."""
                    t0 = c * XTC
                    xld = xldp.tile([128, 512], F32, tag="xld", name="xld")
                    xlb = xldp.tile([128, 512], BF16, tag="xlb", name="xlb")
                    nc.sync.dma_start(xld[:], x_d[b, t0 : t0 + XTC, :])
                    nc.vector.tensor_copy(xlb[:], xld[:])
                    # x^T via DMA transpose: xT[p, ic, t] = x[t, ic*128+p]
                    xT = xtrp.tile([128, KT * XTC], BF16, tag="xT", name="xT")
                    xTv = xT[:].rearrange("p (ic t) -> p ic t", ic=KT)
                    nc.sync.dma_start_transpose(xTv[:, :, :], xlb[:])
                    for hc in range(MT):
                        ps = psp.tile(
                            [128, XTC], F32, tag=ptag, name="xps", bufs=pbufs
                        )
                        for ic in range(KT):
                            j = (ic * MT + hc) * 128
                            nc.tensor.matmul(
                                ps[:],
                                wih_T[:, j : j + 128],
                                xTv[:, ic, :],
                                start=(ic == 0),
                                stop=(ic == KT - 1),
                            )
                        nc.scalar.add(
                            xp_v[:, t0 : t0 + XTC, hc, b],
                            ps[:],
                            bias_t[:, hc : hc + 1],
                        )

                # prologue: chunk c=0 for all batch rows, before the loop
                with tc.tile_pool(name="xpp0", bufs=4, space="PSUM") as xpp0:
                    for b in range(BL):
                        xproj_block(b, 0, xpp0, "xps0", None)

                # ========== the recurrence ==========
                with (
                    tc.tile_pool(name="stg", bufs=NW + 1) as stgp,
                    tc.tile_pool(name="rec", bufs=2, space="PSUM") as recp,
                    tc.tile_pool(name="xpp", bufs=1, space="PSUM") as xppp,
                    tc.tile_pool(name="otb", bufs=3) as otbp,
                    tc.tile_pool(name="owd", bufs=3) as owdp,
                ):
                    stag = [
                        stgp.tile(
                            [128, WIN * MT * BL], BF16, tag="stg", name=f"st{w}"
                        )
                        for w in range(NW)
                    ]
                    xsched = {}
                    for i in range(8 * (NXB - 1)):
                        wa = i // 2 if i < 8 else i - 4
                        xsched.setdefault(wa, []).append((1 + i // 8, i % 8))
                    ow = None
                    for t in range(SL):
                        tl = t % WIN
                        w = t // WIN
                        ptl = (WIN - 1) if tl == 0 else (tl - 1)
                        hprev = (
                            (stag[w - 1] if w > 0 else stag_init)
                            if tl == 0
                            else stag[w]
                        )
                        sc = stag[w]
                        if tl == 0 and w > 0:
                            ow = owdp.tile([128, H], F32, tag="ow", name="ow")
                        psx = [
                            recp.tile(
                                [128, BL], F32, tag=f"rec{c}", name=f"ps{c}",
                                bufs=1 if c == 3 else 2,
                            )
                            for c in range(MT)
                        ]

                        # inline finalize of window w-1 (ACT + DMA only)
                        if w > 0 and tl < MT:
                            hc = tl
                            otb = otbp.tile(
                                [128, 128], BF16, tag="otb", name="otb"
                            )
                            nc.sync.dma_start_transpose(
                                otb[:],
                                stag[w - 1][:, hc * WIN * BL :][:, : WIN * BL],
                            )
                            nc.scalar.copy(
                                ow[:, hc * 128 : (hc + 1) * 128], otb[:]
                            )
                        if w > 0 and tl == MT:
                            dst = out_d[
                                :, (w - 1) * WIN : w * WIN, :
                            ].rearrange("b t h -> t b h")
                            nc.sync.dma_start(dst, ow[:])

                        # Four chunk-groups; rotated k-order per chunk
                        for c in range(MT):
                            o = t * MT * BL + c * BL
                            nc.tensor.matmul(
                                psx[c][:],
                                ident[:],
                                xp_t[:, o : o + BL],
                                start=True,
                                stop=False,
                                skip_group_check=True,
                            )
                            for kk in range(KT):
                                k = (c + kk) % KT
                                j = (k * MT + c) * 128
                                oh = k * WIN * BL + ptl * BL
                                nc.tensor.matmul(
                                    psx[c][:],
                                    whh_T[:, j : j + 128],
                                    hprev[:, oh : oh + BL],
                                    start=False,
                                    stop=(kk == KT - 1),
                                    skip_group_check=True,
                                )
                            dst = sc[:, c * WIN * BL + tl * BL :][:, :BL]
                            nc.vector.tensor_scalar_max(dst, psx[c][:], 0.0)

                        # interleaved x_proj blocks, emitted after the
                        # window's steps -> PE gap filler.  Chunk c's blocks
                        # are spread over earlier windows with >=4 windows of
                        # margin before step c*XTC consumes them.
                        if tl == WIN - 1:
                            for (cc, bb) in xsched.get(w, ()):
                                xproj_block(bb, cc, xppp, "xps", None)

                    # ===== tail: finalize the last window + h_final =====
                    with tc.tile_pool(name="ot2", bufs=4) as ot2p:
                        ow2 = owdp.tile([128, H], F32, tag="ow", name="ow2")
                        for hc in range(MT):
                            otb2 = ot2p.tile(
                                [128, 128], BF16, tag="otb2", name="otb2"
                            )
                            nc.sync.dma_start_transpose(
                                otb2[:],
                                stag[NW - 1][:, hc * WIN * BL :][:, : WIN * BL],
                            )
                            if hc % 2 == 0:
                                nc.vector.tensor_copy(
                                    ow2[:, hc * 128 : (hc + 1) * 128], otb2[:]
                                )
                            else:
                                nc.scalar.copy(
                                    ow2[:, hc * 128 : (hc + 1) * 128], otb2[:]
                                )
                        dst = out_d[:, (NW - 1) * WIN : NW * WIN, :].rearrange(
                            "b t h -> t b h"
                        )
                        nc.sync.dma_start(dst, ow2[:])
                        nc.sync.dma_start(
                            hf_d[:, :], ow2[(WIN - 1) * BL : WIN * BL, :]
                        )

    _split_multi_waits(nc)
    return nc


_NC_CACHE = {}


def get_nc(seq_len=S):
    if seq_len not in _NC_CACHE:
        _NC_CACHE[seq_len] = build_rnn_kernel(seq_len)
    return _NC_CACHE[seq_len]


def make_in_maps(inputs, h0, weight_ih, weight_hh, bias_ih, bias_hh):
    inputs = np.ascontiguousarray(np.asarray(inputs, dtype=np.float32))
    h0 = np.asarray(h0, dtype=np.float32).reshape(-1, H)
    weight_ih = np.ascontiguousarray(np.asarray(weight_ih, dtype=np.float32))
    weight_hh = np.ascontiguousarray(np.asarray(weight_hh, dtype=np.float32))
    bias_ih = np.ascontiguousarray(np.asarray(bias_ih, dtype=np.float32))
    bias_hh = np.ascontiguousarray(np.asarray(bias_hh, dtype=np.float32))
    in_maps = []
    for c in range(NCORES):
        sl = slice(c * BL, (c + 1) * BL)
        in_maps.append(
            {
                "inputs": np.ascontiguousarray(inputs[sl]),
                "h0": np.ascontiguousarray(h0[sl]),
                "weight_ih": weight_ih,
                "weight_hh": weight_hh,
                "bias_ih": bias_ih,
                "bias_hh": bias_hh,
            }
        )
    return in_maps


def assemble(results, seq_len=S):
    outputs = np.empty((B, seq_len, H), dtype=np.float32)
    h_final = np.empty((B, H), dtype=np.float32)
    for c in range(NCORES):
        sl = slice(c * BL, (c + 1) * BL)
        outputs[sl] = results[c]["outputs"]
        h_final[sl] = results[c]["h_final"]
    return outputs, h_final[None, :, :]


def kernel(inputs, h0, weight_ih, weight_hh, bias_ih, bias_hh):
    nc = get_nc(S)
    in_maps = make_in_maps(inputs, h0, weight_ih, weight_hh, bias_ih, bias_hh)
    res = run_bass_kernel_spmd(nc, in_maps, core_ids=list(range(NCORES)))
    return assemble(res.results, S)


# revision 18
# speedup vs baseline: 3.7216x; 3.7216x over previous
"""Trainium2 Bass kernel for the ReLU-RNN problem (nn_RNN).

Math (per core, data-parallel over batch; 8 batch rows per core):
    xp  = x @ W_ih.T + b_ih + b_hh
    h_t = relu(xp_t + h_{t-1} @ W_hh.T)     (S sequential steps)

All recurrent state lives transposed: h^T is [H=512 (4 partition
chunks), B_local=8].  Each step runs 4 chunk-groups: an identity
matmul injects xp_t into a dedicated PSUM bank, 4 bf16 W_hh tile
matmuls accumulate in ROTATED k-order (chunk c starts at k=c, which
makes the relu-latency constraint uniform across chunks), and one DVE
max writes the bf16 h^T staging for the next step.

The input projection (x_proj) is interleaved into the loop: one
(batch-row, 128-timestep) block per 16-step window, emitted after the
window so the Tile scheduler uses it as PE gap-filler while the
recurrence waits on relu semaphores.  x^T tiles come from DMA
transpose, not the PE.  Output windows are finalized inline the same
way (DMA transpose + ScalarE cast + DMA), so nothing interrupts the
steady state.
"""

import numpy as np

import concourse.bass as bass
import concourse.mybir as mybir
from concourse import masks, tile
from concourse.bass_utils import run_bass_kernel_spmd

B, S, I, H = 64, 1024, 512, 512
NCORES = 8
BL = B // NCORES          # batch rows per core
KT = I // 128             # contraction tiles (4)
MT = H // 128             # output-row tiles (4)
WIN = 16                  # recurrence steps per output window
XTC = 128                 # x_proj timesteps per interleaved block
F32 = mybir.dt.float32
BF16 = mybir.dt.bfloat16

# This walrus build rejects instructions carrying more than one sync-wait
# command.  After Tile scheduling, split any excess waits onto same-engine
# NOP instructions inserted immediately before the offending instruction.
_MAX_WAITS = 1


def _split_multi_waits(nc):
    cnt = 0
    for bb in nc.main_func.blocks:
        insts = bb.instructions
        i = 0
        while i < len(insts):
            inst = insts[i]
            si = getattr(inst, "sync_info", None)
            if si is not None and len(si.on_wait) > _MAX_WAITS:
                waits = list(si.on_wait)
                si.on_wait = waits[-_MAX_WAITS:]
                rest = waits[:-_MAX_WAITS]
                for j in range(0, len(rest), _MAX_WAITS):
                    nop = mybir.InstNoOp(
                        name=f"ws_{cnt}",
                        engine=inst.engine,
                        sync_info=mybir.SyncInfo(
                            on_wait=rest[j : j + _MAX_WAITS], on_update=[]
                        ),
                        bass_nofuse=True,
                    )
                    cnt += 1
                    insts.insert(i, nop)
                    i += 1
            i += 1
    return cnt


def build_rnn_kernel(seq_len=S):
    SL = seq_len
    NW = SL // WIN
    NXB = SL // XTC          # x_proj 128-step chunks
    assert SL % WIN == 0 and SL % XTC == 0
    nc = bass.Bass(target_bir_lowering=False, trn_type="TRN2")

    x_d = nc.dram_tensor("inputs", [BL, SL, I], F32, kind="ExternalInput")
    h0_d = nc.dram_tensor("h0", [BL, H], F32, kind="ExternalInput")
    wih_d = nc.dram_tensor("weight_ih", [H, I], F32, kind="ExternalInput")
    whh_d = nc.dram_tensor("weight_hh", [H, H], F32, kind="ExternalInput")
    bih_d = nc.dram_tensor("bias_ih", [H], F32, kind="ExternalInput")
    bhh_d = nc.dram_tensor("bias_hh", [H], F32, kind="ExternalInput")
    out_d = nc.dram_tensor("outputs", [BL, SL, H], F32, kind="ExternalOutput")
    hf_d = nc.dram_tensor("h_final", [BL, H], F32, kind="ExternalOutput")

    with tile.TileContext(nc) as tc:
        with tc.tile_pool(name="const", bufs=1) as constp:
            ident = constp.tile([128, 128], BF16)
            masks.make_identity(nc, ident[:])

            # bias_t[p, hc] = (b_ih + b_hh)[hc*128 + p]
            bias_t = constp.tile([128, MT], F32)
            btmp = constp.tile([128, MT], F32)
            nc.sync.dma_start(bias_t[:], bih_d.rearrange("(hc p) -> p hc", p=128))
            nc.sync.dma_start(btmp[:], bhh_d.rearrange("(hc p) -> p hc", p=128))
            nc.vector.tensor_add(bias_t[:], bias_t[:], btmp[:])

            # lhsT weight tiles: lhsT(kc, mc)[k, m] = W[mc*128+m, kc*128+k]
            wih_T = constp.tile([128, KT * MT * 128], BF16)   # col idx (kc*MT+mc)
            whh_T = constp.tile([128, KT * MT * 128], BF16)
            # xp^T, SBUF-resident bf16, col idx (t, hc, b)
            xp_t = constp.tile([128, SL * MT * BL], BF16)
            # initial h^T staging window (holds h0 in slot WIN-1), idx (hc,t,b)
            stag_init = constp.tile([128, WIN * MT * BL], BF16)

            with (
                tc.tile_pool(name="wld", bufs=2) as wldp,
                tc.tile_pool(name="wtp", bufs=2, space="PSUM") as wtpp,
            ):
                for w_d, w_T in ((wih_d, wih_T), (whh_d, whh_T)):
                    for mc in range(MT):
                        wld = wldp.tile([128, 512], F32, tag="wld")
                        wldb = wldp.tile([128, 512], BF16, tag="wldb")
                        nc.sync.dma_start(wld[:], w_d[mc * 128 : (mc + 1) * 128, :])
                        nc.vector.tensor_copy(wldb[:], wld[:])
                        for kc in range(KT):
                            ps = wtpp.tile([128, 128], BF16, tag="wtp")
                            nc.tensor.transpose(
                                ps[:], wldb[:, kc * 128 : (kc + 1) * 128], ident[:]
                            )
                            j = (kc * MT + mc) * 128
                            nc.vector.tensor_copy(w_T[:, j : j + 128], ps[:])

                # h0 -> bf16, transposed into stag_init slot WIN-1
                h0l = wldp.tile([BL, H], F32, tag="h0l")
                h0b = wldp.tile([BL, H], BF16, tag="h0b")
                nc.sync.dma_start(h0l[:], h0_d[:, :])
                nc.vector.tensor_copy(h0b[:], h0l[:])
                for hc in range(MT):
                    ps = wtpp.tile([128, 128], BF16, tag="wtp")
                    nc.tensor.transpose(
                        ps[:, :BL], h0b[:, hc * 128 : (hc + 1) * 128],
                        ident[:BL, :BL],
                    )
                    off = hc * WIN * BL + (WIN - 1) * BL
                    nc.vector.tensor_copy(
                        stag_init[:, off : off + BL], ps[:, :BL]
                    )

            # view of xp^T as [p, t, hc, b]
            xp_v = xp_t[:].rearrange("p (t hc b) -> p t hc b", hc=MT, b=BL)

            with (
                tc.tile_pool(name="xld", bufs=3) as xldp,
                tc.tile_pool(name="xtr", bufs=3) as xtrp,
            ):
                def xproj_block(b, c, psp, ptag, pbufs):
                    """x_proj for batch row b, timesteps [c*XTC, (c+1)*XTC)."""
                    t0 = c * XTC
                    xld = xldp.tile([128, 512], F32, tag="xld", name="xld")
                    xlb = xldp.tile([128, 512], BF16, tag="xlb", name="xlb")
                    nc.sync.dma_start(xld[:], x_d[b, t0 : t0 + XTC, :])
                    nc.scalar.copy(xlb[:], xld[:])
                    # x^T via DMA transpose: xT[p, ic, t] = x[t, ic*128+p]
                    xT = xtrp.tile([128, KT * XTC], BF16, tag="xT", name="xT")
                    xTv = xT[:].rearrange("p (ic t) -> p ic t", ic=KT)
                    nc.sync.dma_start_transpose(xTv[:, :, :], xlb[:])
                    for hc in range(MT):
                        ps = psp.tile(
                            [128, XTC], F32, tag=ptag, name="xps", bufs=pbufs
                        )
                        for ic in range(KT):
                            j = (ic * MT + hc) * 128
                            nc.tensor.matmul(
                                ps[:],
                                wih_T[:, j : j + 128],
                                xTv[:, ic, :],
                                start=(ic == 0),
                                stop=(ic == KT - 1),
                            )
                        nc.scalar.add(
                            xp_v[:, t0 : t0 + XTC, hc, b],
                            ps[:],
                            bias_t[:, hc : hc + 1],
                        )

                # prologue: chunk c=0 for all batch rows, before the loop
                with tc.tile_pool(name="xpp0", bufs=4, space="PSUM") as xpp0:
                    for b in range(BL):
                        xproj_block(b, 0, xpp0, "xps0", None)

                # ========== the recurrence ==========
                with (
                    tc.tile_pool(name="stg", bufs=NW + 1) as stgp,
                    tc.tile_pool(name="rec", bufs=2, space="PSUM") as recp,
                    tc.tile_pool(name="xpp", bufs=1, space="PSUM") as xppp,
                    tc.tile_pool(name="otb", bufs=3) as otbp,
                    tc.tile_pool(name="owd", bufs=3) as owdp,
                ):
                    stag = [
                        stgp.tile(
                            [128, WIN * MT * BL], BF16, tag="stg", name=f"st{w}"
                        )
                        for w in range(NW)
                    ]
                    xsched = {}
                    for i in range(8 * (NXB - 1)):
                        wa = i // 2 if i < 8 else i - 4
                        xsched.setdefault(wa, []).append((1 + i // 8, i % 8))
                    ow = None
                    for t in range(SL):
                        tl = t % WIN
                        w = t // WIN
                        ptl = (WIN - 1) if tl == 0 else (tl - 1)
                        hprev = (
                            (stag[w - 1] if w > 0 else stag_init)
                            if tl == 0
                            else stag[w]
                        )
                        sc = stag[w]
                        if tl == 0 and w > 0:
                            ow = owdp.tile([128, H], F32, tag="ow", name="ow")
                        psx = [
                            recp.tile(
                                [128, BL], F32, tag=f"rec{c}", name=f"ps{c}",
                                bufs=1 if c == 3 else 2,
                            )
                            for c in range(MT)
                        ]

                        # inline finalize of window w-1 (ACT + DMA only)
                        if w > 0 and tl < MT:
                            hc = tl
                            otb = otbp.tile(
                                [128, 128], BF16, tag="otb", name="otb"
                            )
                            nc.sync.dma_start_transpose(
                                otb[:],
                                stag[w - 1][:, hc * WIN * BL :][:, : WIN * BL],
                            )
                            nc.scalar.copy(
                                ow[:, hc * 128 : (hc + 1) * 128], otb[:]
                            )
                        if w > 0 and tl == MT:
                            dst = out_d[
                                :, (w - 1) * WIN : w * WIN, :
                            ].rearrange("b t h -> t b h")
                            nc.sync.dma_start(dst, ow[:])

                        # Four chunk-groups; rotated k-order per chunk
                        for c in range(MT):
                            o = t * MT * BL + c * BL
                            nc.tensor.matmul(
                                psx[c][:],
                                ident[:],
                                xp_t[:, o : o + BL],
                                start=True,
                                stop=False,
                                skip_group_check=True,
                            )
                            for kk in range(KT):
                                k = (c + kk) % KT
                                j = (k * MT + c) * 128
                                oh = k * WIN * BL + ptl * BL
                                nc.tensor.matmul(
                                    psx[c][:],
                                    whh_T[:, j : j + 128],
                                    hprev[:, oh : oh + BL],
                                    start=False,
                                    stop=(kk == KT - 1),
                                    skip_group_check=True,
                                )
                            dst = sc[:, c * WIN * BL + tl * BL :][:, :BL]
                            nc.vector.tensor_scalar_max(dst, psx[c][:], 0.0)

                        # interleaved x_proj blocks, emitted after the
                        # window's steps -> PE gap filler.  Chunk c's blocks
                        # are spread over earlier windows with >=4 windows of
                        # margin before step c*XTC consumes them.
                        if tl == WIN - 1:
                            blocks = xsched.get(w, ())
                            if blocks:
                                # gap-filler priority: the scheduler should
                                # only run these when recurrence work isn't
                                # ready
                                tc.cur_priority += 1_000_000
                                for (cc, bb) in blocks:
                                    xproj_block(bb, cc, xppp, "xps", None)
                                tc.cur_priority -= 1_000_000

                    # ===== tail: finalize the last window + h_final =====
                    with tc.tile_pool(name="ot2", bufs=4) as ot2p:
                        ow2 = owdp.tile([128, H], F32, tag="ow", name="ow2")
                        for hc in range(MT):
                            otb2 = ot2p.tile(
                                [128, 128], BF16, tag="otb2", name="otb2"
                            )
                            nc.sync.dma_start_transpose(
                                otb2[:],
                                stag[NW - 1][:, hc * WIN * BL :][:, : WIN * BL],
                            )
                            if hc % 2 == 0:
                                nc.vector.tensor_copy(
                                    ow2[:, hc * 128 : (hc + 1) * 128], otb2[:]
                                )
                            else:
                                nc.scalar.copy(
                                    ow2[:, hc * 128 : (hc + 1) * 128], otb2[:]
                                )
                        dst = out_d[:, (NW - 1) * WIN : NW * WIN, :].rearrange(
                            "b t h -> t b h"
                        )
                        nc.sync.dma_start(dst, ow2[:])
                        nc.sync.dma_start(
                            hf_d[:, :], ow2[(WIN - 1) * BL : WIN * BL, :]
                        )

    _split_multi_waits(nc)
    return nc


_NC_CACHE = {}


def get_nc(seq_len=S):
    if seq_len not in _NC_CACHE:
        _NC_CACHE[seq_len] = build_rnn_kernel(seq_len)
    return _NC_CACHE[seq_len]


def make_in_maps(inputs, h0, weight_ih, weight_hh, bias_ih, bias_hh):
    inputs = np.ascontiguousarray(np.asarray(inputs, dtype=np.float32))
    h0 = np.asarray(h0, dtype=np.float32).reshape(-1, H)
    weight_ih = np.ascontiguousarray(np.asarray(weight_ih, dtype=np.float32))
    weight_hh = np.ascontiguousarray(np.asarray(weight_hh, dtype=np.float32))
    bias_ih = np.ascontiguousarray(np.asarray(bias_ih, dtype=np.float32))
    bias_hh = np.ascontiguousarray(np.asarray(bias_hh, dtype=np.float32))
    in_maps = []
    for c in range(NCORES):
        sl = slice(c * BL, (c + 1) * BL)
        in_maps.append(
            {
                "inputs": np.ascontiguousarray(inputs[sl]),
                "h0": np.ascontiguousarray(h0[sl]),
                "weight_ih": weight_ih,
                "weight_hh": weight_hh,
                "bias_ih": bias_ih,
                "bias_hh": bias_hh,
            }
        )
    return in_maps


def assemble(results, seq_len=S):
    outputs = np.empty((B, seq_len, H), dtype=np.float32)
    h_final = np.empty((B, H), dtype=np.float32)
    for c in range(NCORES):
        sl = slice(c * BL, (c + 1) * BL)
        outputs[sl] = results[c]["outputs"]
        h_final[sl] = results[c]["h_final"]
    return outputs, h_final[None, :, :]


def kernel(inputs, h0, weight_ih, weight_hh, bias_ih, bias_hh):
    nc = get_nc(S)
    in_maps = make_in_maps(inputs, h0, weight_ih, weight_hh, bias_ih, bias_hh)
    res = run_bass_kernel_spmd(nc, in_maps, core_ids=list(range(NCORES)))
    return assemble(res.results, S)


# revision 19
# speedup vs baseline: 3.8225x; 1.0271x over previous
"""Trainium2 Bass kernel for the ReLU-RNN problem (nn_RNN).

Math (per core, data-parallel over batch; 8 batch rows per core):
    xp  = x @ W_ih.T + b_ih + b_hh
    h_t = relu(xp_t + h_{t-1} @ W_hh.T)     (S sequential steps)

All recurrent state lives transposed: h^T is [H=512 (4 partition
chunks), B_local=8].  Each step runs 4 chunk-groups: an identity
matmul injects xp_t into a dedicated PSUM bank, 4 bf16 W_hh tile
matmuls accumulate in ROTATED k-order (chunk c starts at k=c, which
makes the relu-latency constraint uniform across chunks), and one DVE
max writes the bf16 h^T staging for the next step.

The input projection (x_proj) is interleaved into the loop: one
(batch-row, 128-timestep) block per 16-step window, emitted after the
window so the Tile scheduler uses it as PE gap-filler while the
recurrence waits on relu semaphores.  x^T tiles come from DMA
transpose, not the PE.  Output windows are finalized inline the same
way (DMA transpose + ScalarE cast + DMA), so nothing interrupts the
steady state.
"""

import numpy as np

import concourse.bass as bass
import concourse.mybir as mybir
from concourse import masks, tile
from concourse.bass_utils import run_bass_kernel_spmd

B, S, I, H = 64, 1024, 512, 512
NCORES = 8
BL = B // NCORES          # batch rows per core
KT = I // 128             # contraction tiles (4)
MT = H // 128             # output-row tiles (4)
WIN = 16                  # recurrence steps per output window
XTC = 128                 # x_proj timesteps per interleaved block
F32 = mybir.dt.float32
BF16 = mybir.dt.bfloat16

# This walrus build rejects instructions carrying more than one sync-wait
# command.  After Tile scheduling, split any excess waits onto same-engine
# NOP instructions inserted immediately before the offending instruction.
_MAX_WAITS = 1


def _split_multi_waits(nc):
    cnt = 0
    for bb in nc.main_func.blocks:
        insts = bb.instructions
        i = 0
        while i < len(insts):
            inst = insts[i]
            si = getattr(inst, "sync_info", None)
            if si is not None and len(si.on_wait) > _MAX_WAITS:
                waits = list(si.on_wait)
                si.on_wait = waits[-_MAX_WAITS:]
                rest = waits[:-_MAX_WAITS]
                for j in range(0, len(rest), _MAX_WAITS):
                    nop = mybir.InstNoOp(
                        name=f"ws_{cnt}",
                        engine=inst.engine,
                        sync_info=mybir.SyncInfo(
                            on_wait=rest[j : j + _MAX_WAITS], on_update=[]
                        ),
                        bass_nofuse=True,
                    )
                    cnt += 1
                    insts.insert(i, nop)
                    i += 1
            i += 1
    return cnt


def build_rnn_kernel(seq_len=S):
    SL = seq_len
    NW = SL // WIN
    NXB = SL // XTC          # x_proj 128-step chunks
    assert SL % WIN == 0 and SL % XTC == 0
    nc = bass.Bass(target_bir_lowering=False, trn_type="TRN2")

    x_d = nc.dram_tensor("inputs", [BL, SL, I], F32, kind="ExternalInput")
    h0_d = nc.dram_tensor("h0", [BL, H], F32, kind="ExternalInput")
    wih_d = nc.dram_tensor("weight_ih", [H, I], F32, kind="ExternalInput")
    whh_d = nc.dram_tensor("weight_hh", [H, H], F32, kind="ExternalInput")
    bih_d = nc.dram_tensor("bias_ih", [H], F32, kind="ExternalInput")
    bhh_d = nc.dram_tensor("bias_hh", [H], F32, kind="ExternalInput")
    out_d = nc.dram_tensor("outputs", [BL, SL, H], F32, kind="ExternalOutput")
    hf_d = nc.dram_tensor("h_final", [BL, H], F32, kind="ExternalOutput")

    with tile.TileContext(nc) as tc:
        with tc.tile_pool(name="const", bufs=1) as constp:
            ident = constp.tile([128, 128], BF16)
            masks.make_identity(nc, ident[:])

            # bias_t[p, hc] = (b_ih + b_hh)[hc*128 + p]
            bias_t = constp.tile([128, MT], F32)
            btmp = constp.tile([128, MT], F32)
            nc.sync.dma_start(bias_t[:], bih_d.rearrange("(hc p) -> p hc", p=128))
            nc.sync.dma_start(btmp[:], bhh_d.rearrange("(hc p) -> p hc", p=128))
            nc.vector.tensor_add(bias_t[:], bias_t[:], btmp[:])

            # lhsT weight tiles: lhsT(kc, mc)[k, m] = W[mc*128+m, kc*128+k]
            wih_T = constp.tile([128, KT * MT * 128], BF16)   # col idx (kc*MT+mc)
            whh_T = constp.tile([128, KT * MT * 128], BF16)
            # xp^T, SBUF-resident bf16, col idx (t, hc, b)
            xp_t = constp.tile([128, SL * MT * BL], BF16)
            # initial h^T staging window (holds h0 in slot WIN-1), idx (hc,t,b)
            stag_init = constp.tile([128, WIN * MT * BL], BF16)

            with (
                tc.tile_pool(name="wld", bufs=2) as wldp,
                tc.tile_pool(name="wtp", bufs=2, space="PSUM") as wtpp,
            ):
                for w_d, w_T in ((wih_d, wih_T), (whh_d, whh_T)):
                    for mc in range(MT):
                        wld = wldp.tile([128, 512], F32, tag="wld")
                        wldb = wldp.tile([128, 512], BF16, tag="wldb")
                        nc.sync.dma_start(wld[:], w_d[mc * 128 : (mc + 1) * 128, :])
                        nc.vector.tensor_copy(wldb[:], wld[:])
                        for kc in range(KT):
                            ps = wtpp.tile([128, 128], BF16, tag="wtp")
                            nc.tensor.transpose(
                                ps[:], wldb[:, kc * 128 : (kc + 1) * 128], ident[:]
                            )
                            j = (kc * MT + mc) * 128
                            nc.vector.tensor_copy(w_T[:, j : j + 128], ps[:])

                # h0 -> bf16, transposed into stag_init slot WIN-1
                h0l = wldp.tile([BL, H], F32, tag="h0l")
                h0b = wldp.tile([BL, H], BF16, tag="h0b")
                nc.sync.dma_start(h0l[:], h0_d[:, :])
                nc.vector.tensor_copy(h0b[:], h0l[:])
                for hc in range(MT):
                    ps = wtpp.tile([128, 128], BF16, tag="wtp")
                    nc.tensor.transpose(
                        ps[:, :BL], h0b[:, hc * 128 : (hc + 1) * 128],
                        ident[:BL, :BL],
                    )
                    off = hc * WIN * BL + (WIN - 1) * BL
                    nc.vector.tensor_copy(
                        stag_init[:, off : off + BL], ps[:, :BL]
                    )

            # view of xp^T as [p, t, hc, b]
            xp_v = xp_t[:].rearrange("p (t hc b) -> p t hc b", hc=MT, b=BL)

            with (
                tc.tile_pool(name="xld", bufs=3) as xldp,
                tc.tile_pool(name="xtr", bufs=3) as xtrp,
            ):
                def xproj_block(b, c, psp, ptag, pbufs):
                    """x_proj for batch row b, timesteps [c*XTC, (c+1)*XTC)."""
                    t0 = c * XTC
                    xld = xldp.tile([128, 512], F32, tag="xld", name="xld")
                    xlb = xldp.tile([128, 512], BF16, tag="xlb", name="xlb")
                    nc.sync.dma_start(xld[:], x_d[b, t0 : t0 + XTC, :])
                    nc.scalar.copy(xlb[:], xld[:])
                    # x^T via DMA transpose: xT[p, ic, t] = x[t, ic*128+p]
                    xT = xtrp.tile([128, KT * XTC], BF16, tag="xT", name="xT")
                    xTv = xT[:].rearrange("p (ic t) -> p ic t", ic=KT)
                    nc.sync.dma_start_transpose(xTv[:, :, :], xlb[:])
                    for hc in range(MT):
                        ps = psp.tile(
                            [128, XTC], F32, tag=ptag, name="xps", bufs=pbufs
                        )
                        for ic in range(KT):
                            j = (ic * MT + hc) * 128
                            nc.tensor.matmul(
                                ps[:],
                                wih_T[:, j : j + 128],
                                xTv[:, ic, :],
                                start=(ic == 0),
                                stop=(ic == KT - 1),
                            )
                        nc.scalar.add(
                            xp_v[:, t0 : t0 + XTC, hc, b],
                            ps[:],
                            bias_t[:, hc : hc + 1],
                        )

                # prologue: chunk c=0 for all batch rows, before the loop
                with tc.tile_pool(name="xpp0", bufs=4, space="PSUM") as xpp0:
                    for b in range(BL):
                        xproj_block(b, 0, xpp0, "xps0", None)

                # ========== the recurrence ==========
                with (
                    tc.tile_pool(name="stg", bufs=NW + 1) as stgp,
                    tc.tile_pool(name="rec", bufs=2, space="PSUM") as recp,
                    tc.tile_pool(name="xpp", bufs=2, space="PSUM") as xppp,
                    tc.tile_pool(name="otb", bufs=3) as otbp,
                    tc.tile_pool(name="owd", bufs=3) as owdp,
                ):
                    stag = [
                        stgp.tile(
                            [128, WIN * MT * BL], BF16, tag="stg", name=f"st{w}"
                        )
                        for w in range(NW)
                    ]
                    xsched = {}
                    for i in range(8 * (NXB - 1)):
                        wa = i // 2 if i < 8 else i - 4
                        xsched.setdefault(wa, []).append((1 + i // 8, i % 8))
                    ow = None
                    for t in range(SL):
                        tl = t % WIN
                        w = t // WIN
                        ptl = (WIN - 1) if tl == 0 else (tl - 1)
                        hprev = (
                            (stag[w - 1] if w > 0 else stag_init)
                            if tl == 0
                            else stag[w]
                        )
                        sc = stag[w]
                        if tl == 0 and w > 0:
                            ow = owdp.tile([128, H], F32, tag="ow", name="ow")
                        psx = [
                            recp.tile(
                                [128, BL], F32, tag=f"rec{c}", name=f"ps{c}",
                                bufs=1 if c >= 2 else 2,
                            )
                            for c in range(MT)
                        ]

                        # inline finalize of window w-1 (ACT + DMA only)
                        if w > 0 and tl < MT:
                            hc = tl
                            otb = otbp.tile(
                                [128, 128], BF16, tag="otb", name="otb"
                            )
                            nc.sync.dma_start_transpose(
                                otb[:],
                                stag[w - 1][:, hc * WIN * BL :][:, : WIN * BL],
                            )
                            nc.scalar.copy(
                                ow[:, hc * 128 : (hc + 1) * 128], otb[:]
                            )
                        if w > 0 and tl == MT:
                            dst = out_d[
                                :, (w - 1) * WIN : w * WIN, :
                            ].rearrange("b t h -> t b h")
                            nc.sync.dma_start(dst, ow[:])

                        # Four chunk-groups; rotated k-order per chunk
                        for c in range(MT):
                            o = t * MT * BL + c * BL
                            nc.tensor.matmul(
                                psx[c][:],
                                ident[:],
                                xp_t[:, o : o + BL],
                                start=True,
                                stop=False,
                                skip_group_check=True,
                            )
                            for kk in range(KT):
                                k = (c + kk) % KT
                                j = (k * MT + c) * 128
                                oh = k * WIN * BL + ptl * BL
                                nc.tensor.matmul(
                                    psx[c][:],
                                    whh_T[:, j : j + 128],
                                    hprev[:, oh : oh + BL],
                                    start=False,
                                    stop=(kk == KT - 1),
                                    skip_group_check=True,
                                )
                            dst = sc[:, c * WIN * BL + tl * BL :][:, :BL]
                            nc.vector.tensor_scalar_max(dst, psx[c][:], 0.0)

                        # interleaved x_proj blocks, emitted after the
                        # window's steps -> PE gap filler.  Chunk c's blocks
                        # are spread over earlier windows with >=4 windows of
                        # margin before step c*XTC consumes them.
                        if tl == WIN - 1:
                            blocks = xsched.get(w, ())
                            if blocks:
                                # gap-filler priority: the scheduler should
                                # only run these when recurrence work isn't
                                # ready
                                tc.cur_priority += 1_000_000
                                for (cc, bb) in blocks:
                                    xproj_block(bb, cc, xppp, "xps", None)
                                tc.cur_priority -= 1_000_000

                    # ===== tail: finalize the last window + h_final =====
                    with tc.tile_pool(name="ot2", bufs=4) as ot2p:
                        ow2 = owdp.tile([128, H], F32, tag="ow", name="ow2")
                        for hc in range(MT):
                            otb2 = ot2p.tile(
                                [128, 128], BF16, tag="otb2", name="otb2"
                            )
                            nc.sync.dma_start_transpose(
                                otb2[:],
                                stag[NW - 1][:, hc * WIN * BL :][:, : WIN * BL],
                            )
                            if hc % 2 == 0:
                                nc.vector.tensor_copy(
                                    ow2[:, hc * 128 : (hc + 1) * 128], otb2[:]
                                )
                            else:
                                nc.scalar.copy(
                                    ow2[:, hc * 128 : (hc + 1) * 128], otb2[:]
                                )
                        dst = out_d[:, (NW - 1) * WIN : NW * WIN, :].rearrange(
                            "b t h -> t b h"
                        )
                        nc.sync.dma_start(dst, ow2[:])
                        nc.sync.dma_start(
                            hf_d[:, :], ow2[(WIN - 1) * BL : WIN * BL, :]
                        )

    _split_multi_waits(nc)
    return nc


_NC_CACHE = {}


def get_nc(seq_len=S):
    if seq_len not in _NC_CACHE:
        _NC_CACHE[seq_len] = build_rnn_kernel(seq_len)
    return _NC_CACHE[seq_len]


def make_in_maps(inputs, h0, weight_ih, weight_hh, bias_ih, bias_hh):
    inputs = np.ascontiguousarray(np.asarray(inputs, dtype=np.float32))
    h0 = np.asarray(h0, dtype=np.float32).reshape(-1, H)
    weight_ih = np.ascontiguousarray(np.asarray(weight_ih, dtype=np.float32))
    weight_hh = np.ascontiguousarray(np.asarray(weight_hh, dtype=np.float32))
    bias_ih = np.ascontiguousarray(np.asarray(bias_ih, dtype=np.float32))
    bias_hh = np.ascontiguousarray(np.asarray(bias_hh, dtype=np.float32))
    in_maps = []
    for c in range(NCORES):
        sl = slice(c * BL, (c + 1) * BL)
        in_maps.append(
            {
                "inputs": np.ascontiguousarray(inputs[sl]),
                "h0": np.ascontiguousarray(h0[sl]),
                "weight_ih": weight_ih,
                "weight_hh": weight_hh,
                "bias_ih": bias_ih,
                "bias_hh": bias_hh,
            }
        )
    return in_maps


def assemble(results, seq_len=S):
    outputs = np.empty((B, seq_len, H), dtype=np.float32)
    h_final = np.empty((B, H), dtype=np.float32)
    for c in range(NCORES):
        sl = slice(c * BL, (c + 1) * BL)
        outputs[sl] = results[c]["outputs"]
        h_final[sl] = results[c]["h_final"]
    return outputs, h_final[None, :, :]


def kernel(inputs, h0, weight_ih, weight_hh, bias_ih, bias_hh):
    nc = get_nc(S)
    in_maps = make_in_maps(inputs, h0, weight_ih, weight_hh, bias_ih, bias_hh)
    res = run_bass_kernel_spmd(nc, in_maps, core_ids=list(range(NCORES)))
    return assemble(res.results, S)
